# revision 64
# baseline (speedup 1.0000x reference)
"""Exphormer attention (GNN message passing) Trainium2 Bass kernel.

Strategy (dst-sharded, zero collectives):
  - Core m owns nodes [m*12500, (m+1)*12500) and all edges pointing into
    them; each core computes its output slice, no collectives needed.
  - Edges are grouped by (dst-chunk of 128 nodes, src-quarter of the KV
    table), padded to 128-edge subtiles, and ordered (chunk-pair, quarter,
    chunk).  One `dma_gather` call per (chunk-pair, quarter) fetches K|V
    rows for two adjacent groups into one shared kv tile (int16 indices
    local to a 25024-row table quarter, wrapped in 16 partitions; the
    call's tail is trimmed to the worst core's valid count rounded to 16,
    skipping most pad descriptors; output is edge-major: one gathered row
    per partition, one column per 128-index block).
  - One compute wave per call: Ef = eaT @ WE on PE; one-hot
    M[e,n] = (dloc[e]==iota) on DVE; M_T via a partition-broadcast DMA of
    dloc (free-major) + a one-src tensor_scalar eq on DVE; Qd = M_T.T @
    Qchunk on PE per subtile; score = exp(clip(sum_dh K*Ef*Qd)); payload
    [e,72] = [V*score | score]; scatter = payload.T @ M accumulated in
    per-chunk PSUM tiles (two alive per pair).  PSUM operands feed DVE
    directly (separate port from the gather-contended SBUF one) and
    PSUM->SBUF casts ride the scalar (ACT) engine.
  - Chunk epilogue: copy, transpose, out = wV * recip(Z+eps), DMA
    node-major.
"""

import sys

import numpy as np

sys.path.insert(0, "/opt/trn_rl_repo")

import ml_dtypes  # noqa: E402

BF16 = ml_dtypes.bfloat16

# ---------------- problem geometry (hardcoded per contract) ----------------
N = 100000
NE = 1250000
D = 64
H = 8
DH = 8
NCORES = 8
NPC = N // NCORES          # 12500 nodes per core
CHUNK = 128                # nodes per dst-chunk
NCHUNK = (NPC + CHUNK - 1) // CHUNK   # 98
NPAD = NCHUNK * CHUNK      # 12544
NTPAD = 100096             # table rows (4 * QSIZE)
NQ = 4                     # table quarters (int16 gather index range)
QSIZE = NTPAD // NQ        # 25024 rows per quarter (< 32768)
SUB = 128                  # edges per subtile
GCALL_SUB = 8              # max subtiles per call (fw ring: 64 desc/engine)
NO_GATHER = False          # debug: replace gather with memset
EXP_CLIP = 5.0


# ---------------- host-side preprocessing ----------------
def _preprocess(x, edge_attr, WQ, WK, WV, WE, edge_index):
    src = np.ascontiguousarray(edge_index[0]).astype(np.int64)
    dst = np.ascontiguousarray(edge_index[1]).astype(np.int64)
    core_of = dst // NPC
    dloc_all = dst - core_of * NPC
    chunk_all = dloc_all // CHUNK
    quarter_all = src // QSIZE

    # order edges by (core, chunk, quarter)
    order = np.lexsort((quarter_all, chunk_all, core_of))
    src_s = src[order]
    dloc_s = (dloc_all - chunk_all * CHUNK)[order]
    key_s = (core_of * (NCHUNK * NQ) + chunk_all * NQ + quarter_all)[order]

    # counts per (core, chunk, quarter)
    cnt = np.bincount(key_s, minlength=NCORES * NCHUNK * NQ).reshape(
        NCORES, NCHUNK, NQ)
    # uniform subtile counts per (chunk, quarter): max over cores
    S = np.ceil(cnt.max(axis=0) / SUB).astype(np.int64)     # [NCHUNK, NQ]
    ts = int(S.sum())

    # group table (static program structure): one entry per (c, q) with
    # S>0, ordered (chunk-pair, quarter, chunk) so a (pair, quarter) call
    # covers two adjacent groups
    groups = []           # (c, q, s_count, sub_start)
    sub_start = 0
    for p in range(0, NCHUNK, 2):
        for q in range(NQ):
            for c in (p, p + 1):
                if c >= NCHUNK:
                    continue
                s = int(S[c, q])
                if s == 0:
                    continue
                groups.append((c, q, s, sub_start))
                sub_start += s
    assert sub_start == ts

    # one gather call per group, statically sized to the worst core's
    # valid count rounded up to 16 (idx wrap granularity) — skips most
    # of the pad-to-128 descriptors without any register plumbing
    maxcnt = cnt.max(axis=0)                                 # [NCHUNK, NQ]
    nig = []                                                 # per group
    for (c, q, s, st) in groups:
        nig.append(int(min(s * SUB,
                           (max(int(maxcnt[c, q]), 1) + 15) // 16 * 16)))

    # calls: one per (chunk-pair, quarter).  Interior groups stay padded
    # to s*128 (slot alignment); the call's last group trims to its nig.
    calls = []            # (q, [gi...], n_idx)
    i = 0
    while i < len(groups):
        c, q, s, st = groups[i]
        gl = [i]
        if i + 1 < len(groups):
            c2, q2 = groups[i + 1][0], groups[i + 1][1]
            if q2 == q and c2 // 2 == c // 2:
                gl.append(i + 1)
        n_idx = sum(groups[g][2] * SUB for g in gl[:-1]) + nig[gl[-1]]
        calls.append((q, gl, n_idx))
        i = gl[-1] + 1
    idx_cols = [n // 16 for (q, gl, n) in calls]
    idx_col_start = np.concatenate([[0], np.cumsum(idx_cols)]).astype(int)
    total_idx_cols = int(idx_col_start[-1])

    geom = dict(ts=ts, groups=groups, nig=nig,
                calls=[(q, tuple(gl), n) for (q, gl, n) in calls],
                idx_col_start=idx_col_start,
                total_idx_cols=total_idx_cols, S=S)

    # ---- per-core data staging ----
    core_starts = np.searchsorted(key_s // (NCHUNK * NQ), np.arange(NCORES + 1))
    per_core = []
    for m in range(NCORES):
        lo, hi = core_starts[m], core_starts[m + 1]
        k_loc = key_s[lo:hi] - m * (NCHUNK * NQ)     # chunk*NQ + quarter
        c_src = src_s[lo:hi]
        c_dloc = dloc_s[lo:hi]
        c_eid = order[lo:hi]
        grp_starts = np.searchsorted(k_loc, np.arange(NCHUNK * NQ + 1))

        E_pad = ts * SUB
        srcq_pad = np.zeros(E_pad, dtype=np.int16)   # quarter-local idx
        dloc_pad = np.full(E_pad, -1.0, dtype=np.float32)
        eid_pad = np.full(E_pad, -1, dtype=np.int64)
        for gi, (c, q, s, st) in enumerate(groups):
            a, b = grp_starts[c * NQ + q], grp_starts[c * NQ + q + 1]
            n = b - a
            pos = st * SUB
            srcq_pad[pos:pos + n] = (c_src[a:b] - q * QSIZE).astype(np.int16)
            dloc_pad[pos:pos + n] = c_dloc[a:b]
            eid_pad[pos:pos + n] = c_eid[a:b]

        valid = eid_pad >= 0
        ea_pad = np.zeros((E_pad, D), dtype=BF16)
        ea_pad[valid] = edge_attr[eid_pad[valid]].astype(BF16)
        # [64, ts*128]: per-subtile feature-major, contiguous per group
        eat = np.ascontiguousarray(
            ea_pad.reshape(ts, SUB, D).transpose(2, 0, 1).reshape(
                D, ts * SUB))
        dlcol = np.ascontiguousarray(
            dloc_pad.reshape(ts, SUB).T).astype(BF16)            # [128,ts]
        dlb = dloc_pad.reshape(1, ts * SUB).astype(BF16)         # [1,ts*128]

        # gather idx stream per call: wrapped in 16 partitions,
        # replicated to all 128: idxarr[16g + j, col] = stream pos col*16+j.
        idxarr = np.zeros((128, total_idx_cols), dtype=np.int16)
        for ci, (q, gl, n_idx) in enumerate(calls):
            col0 = int(idx_col_start[ci])
            parts = []
            for k, gi in enumerate(gl):
                st_g, s_g = groups[gi][3], groups[gi][2]
                take = nig[gi] if k == len(gl) - 1 else s_g * SUB
                parts.append(srcq_pad[st_g * SUB:st_g * SUB + take])
            stream = np.concatenate(parts)
            assert stream.size == n_idx
            wrapped = stream.reshape(-1, 16).T                   # [16, cols]
            idxarr[:, col0:col0 + wrapped.shape[1]] = np.tile(wrapped, (8, 1))

        n0 = m * NPC
        xq = np.zeros((NPAD, D), dtype=np.float32)
        xq[:NPC] = x[n0:n0 + NPC]
        xtq = np.ascontiguousarray(xq.T).astype(BF16)

        per_core.append(dict(eat=eat, dlcol=dlcol, dlb=dlb, idxarr=idxarr,
                             xtq=xtq))

    xt_full = np.zeros((D, NTPAD), dtype=BF16)
    xt_full[:, :N] = x.T.astype(BF16)
    wkv = np.concatenate([WK, WV], axis=1).astype(BF16)
    wq = (WQ / np.sqrt(DH)).astype(BF16)
    we = WE.astype(BF16)
    iota_row = np.ascontiguousarray(np.broadcast_to(
        np.arange(CHUNK, dtype=np.float32), (128, CHUNK)).astype(BF16))
    iota_col = np.arange(128, dtype=np.float32).reshape(128, 1)

    shared = dict(xt=xt_full, wkv=wkv, wq=wq, we=we, iota_row=iota_row,
                  iota_col=iota_col)
    return per_core, shared, geom


# ---------------- device program ----------------
def _build_program(geom):
    from contextlib import ExitStack

    from concourse import bacc, mybir
    import concourse.tile as tile
    from concourse.masks import make_identity
    from concourse.tile_rust import add_dep_helper

    n_table_rows = NTPAD
    nchunk = NCHUNK
    ts = geom["ts"]
    groups = geom["groups"]
    calls = geom["calls"]
    idx_col_start = geom["idx_col_start"]
    total_idx_cols = geom["total_idx_cols"]
    S_MAX = max(g[2] for g in groups)

    dt = mybir.dt
    nc = bacc.Bacc("TRN2", target_bir_lowering=False, debug=False,
                   num_devices=NCORES, dynamic_dma_scratch_size=32768)

    xt = nc.dram_tensor("xt", [D, n_table_rows], dt.bfloat16,
                        kind="ExternalInput").ap()
    xtq = nc.dram_tensor("xtq", [D, nchunk * CHUNK], dt.bfloat16,
                         kind="ExternalInput").ap()
    wkv_d = nc.dram_tensor("wkv", [D, 2 * D], dt.bfloat16,
                           kind="ExternalInput").ap()
    wq_d = nc.dram_tensor("wq", [D, D], dt.bfloat16, kind="ExternalInput").ap()
    we_d = nc.dram_tensor("we", [D, D], dt.bfloat16, kind="ExternalInput").ap()
    iota_d = nc.dram_tensor("iota_row", [128, CHUNK], dt.bfloat16,
                            kind="ExternalInput").ap()
    iotac_d = nc.dram_tensor("iota_col", [128, 1], dt.float32,
                             kind="ExternalInput").ap()
    eat_d = nc.dram_tensor("eat", [D, ts * SUB], dt.bfloat16,
                           kind="ExternalInput").ap()
    idx_d = nc.dram_tensor("idxarr", [128, total_idx_cols], dt.int16,
                           kind="ExternalInput").ap()
    dlcol_d = nc.dram_tensor("dlcol", [128, ts], dt.bfloat16,
                             kind="ExternalInput").ap()
    dlb_d = nc.dram_tensor("dlb", [1, ts * SUB], dt.bfloat16,
                           kind="ExternalInput").ap()
    out_d = nc.dram_tensor("out", [nchunk * CHUNK, D], dt.float32,
                           kind="ExternalOutput").ap()
    kvtab = nc.dram_tensor("kvtab", [n_table_rows, 2 * D], dt.bfloat16).ap()

    with tile.TileContext(nc) as tc, ExitStack() as ctx:
        const_p = ctx.enter_context(tc.tile_pool(name="const", bufs=1))
        sb = ctx.enter_context(tc.tile_pool(name="sb", bufs=4))
        sb2 = ctx.enter_context(tc.tile_pool(name="sb2", bufs=2))
        gat = ctx.enter_context(tc.tile_pool(name="gat", bufs=4))
        ps = ctx.enter_context(tc.tile_pool(name="ps", bufs=1, space="PSUM"))
        ps1 = ctx.enter_context(tc.tile_pool(name="ps1", bufs=1, space="PSUM"))
        ps_acc = ctx.enter_context(
            tc.tile_pool(name="ps_acc", bufs=1, space="PSUM"))

        ident = const_p.tile([128, 128], dt.float32)
        make_identity(nc, ident[:])
        wkv_t = const_p.tile([D, 2 * D], dt.bfloat16)
        nc.sync.dma_start(out=wkv_t[:], in_=wkv_d)
        wq_t = const_p.tile([D, D], dt.bfloat16)
        nc.sync.dma_start(out=wq_t[:], in_=wq_d)
        we_t = const_p.tile([D, D], dt.bfloat16)
        nc.sync.dma_start(out=we_t[:], in_=we_d)
        iota_t = const_p.tile([128, CHUNK], dt.bfloat16)
        nc.sync.dma_start(out=iota_t[:], in_=iota_d)
        iotac_t = const_p.tile([128, 1], dt.float32)
        nc.sync.dma_start(out=iotac_t[:], in_=iotac_d)

        # ---- pre-pass 1: Q table resident in SBUF (first, so the main
        # waves aren't gated on the KV-table build) ----
        qtab = const_p.tile([128, nchunk, D], dt.bfloat16)
        for c0 in range(0, nchunk, 4):
            nq_blk = min(4, nchunk - c0)
            xq_t = sb.tile([D, 4 * 128], dt.bfloat16, tag="xq_t")
            nc.sync.dma_start(out=xq_t[:, 0:nq_blk * 128],
                              in_=xtq[:, c0 * 128:(c0 + nq_blk) * 128])
            for bi in range(nq_blk):
                c = c0 + bi
                q_ps = ps.tile([128, D], dt.float32, name="q_ps",
                               tag="qd" if c % 2 == 0 else "ef")
                nc.tensor.matmul(out=q_ps[:],
                                 lhsT=xq_t[:, bi * 128:(bi + 1) * 128],
                                 rhs=wq_t[:], start=True, stop=True)
                nc.scalar.copy(out=qtab[:, c, :], in_=q_ps[:])

        # ---- pre-pass 2: KV table -> DRAM (8 blocks per iteration,
        # 2-block psum tiles halve the drain copies; gathers gate on the
        # store covering their quarter) ----
        n_tb = n_table_rows // 128
        stores = []
        for b0 in range(0, n_tb, 8):
            nblk = min(8, n_tb - b0)
            xt_t = sb.tile([D, 8 * 128], dt.bfloat16, tag="xt_t")
            nc.sync.dma_start(out=xt_t[:, 0:nblk * 128],
                              in_=xt[:, b0 * 128:(b0 + nblk) * 128])
            kv_sb = sb.tile([128, 8, 2 * D], dt.bfloat16, tag="kv_sb")
            for h in range((nblk + 1) // 2):
                nh = min(2, nblk - 2 * h)
                kv_ps = ps.tile([128, 2, 2 * D], dt.float32, name="kv_ps",
                                tag="ef" if h % 2 == 0 else "qd")
                for k in range(nh):
                    bi = 2 * h + k
                    nc.tensor.matmul(out=kv_ps[:, k, :],
                                     lhsT=xt_t[:, bi * 128:(bi + 1) * 128],
                                     rhs=wkv_t[:], start=True, stop=True)
                nc.scalar.copy(out=kv_sb[:, 2 * h:2 * h + nh, :],
                               in_=kv_ps[:, 0:nh, :])
            stores.append(nc.sync.dma_start(
                out=kvtab[b0 * 128:(b0 + nblk) * 128, :].rearrange(
                    "(blk p) d -> p blk d", p=128),
                in_=kv_sb[:, 0:nblk, :]))
        # store index whose coverage reaches each quarter's end
        qgate = [min((QSIZE * (qq + 1) + 1023) // 1024 - 1, len(stores) - 1)
                 for qq in range(NQ)]

        # ---- main loop: one kv tile per chunk-pair, one gather call and
        # one compute wave per (pair, quarter) ----
        pair_of_call = [groups[gl[0]][0] // 2 for (q, gl, n) in calls]
        pair_calls = {}
        for ci, p in enumerate(pair_of_call):
            pair_calls.setdefault(p, []).append(ci)
        pair_list = sorted(pair_calls)
        pair_idx = {p: i for i, p in enumerate(pair_list)}
        pair_st0 = {p: groups[calls[pair_calls[p][0]][1][0]][3]
                    for p in pair_list}
        SP_MAX = max(sum(groups[gi][2] for ci in pair_calls[p]
                         for gi in calls[ci][1]) for p in pair_list)
        SW_MAX = max(sum(groups[gi][2] for gi in gl) for (q, gl, n) in calls)
        S_CAP = SW_MAX

        first_grp = {}
        last_grp = {}
        for gi, (c, q, s, st) in enumerate(groups):
            if c not in first_grp:
                first_grp[c] = gi
            last_grp[c] = gi

        pair_tiles = {}

        def issue_pair(pi):
            p = pair_list[pi]
            st0 = pair_st0[p]
            kv_t = gat.tile([128, SP_MAX, 2 * D], dt.bfloat16, tag="kvq")
            if pi < 4:
                # first pool rotations: clear garbage (NaN-safety); later
                # rotations inherit finite stale values.  ACT engine: idle
                # during startup, keeps DVE off the critical path
                nc.scalar.memzero(kv_t[:])
            for ci in pair_calls[p]:
                q, gl, n_idx = calls[ci]
                off = groups[gl[0]][3] - st0
                s_call = sum(groups[gi][2] for gi in gl)
                col0 = int(idx_col_start[ci])
                col1 = int(idx_col_start[ci + 1])
                idx_t = sb2.tile([128, S_CAP * 8], dt.int16, tag="idx")
                nc.sync.dma_start(out=idx_t[:, 0:col1 - col0],
                                  in_=idx_d[:, col0:col1])
                g = nc.gpsimd.dma_gather(
                    out_ap=kv_t[:, off:off + s_call, :],
                    in_ap=kvtab[q * QSIZE:(q + 1) * QSIZE, :],
                    idxs_ap=idx_t[:, 0:col1 - col0],
                    num_idxs=n_idx,
                    num_idxs_reg=n_idx,
                    elem_size=2 * D,
                )
                add_dep_helper(g.ins, stores[qgate[q]].ins, True,
                               "gather after its kv quarter is built")
                if qgate[q] > 0:
                    # also gate on the preceding store: covers completion
                    # skew between in-flight store transfers
                    add_dep_helper(g.ins, stores[qgate[q] - 1].ins, True,
                                   "gather after prior kv store")
            pair_tiles[pi] = kv_t

        issue_pair(0)
        next_p = 1
        acc_tiles = {}
        for ci, (wq, gis, n_idx) in enumerate(calls):
            pi = pair_idx[pair_of_call[ci]]
            while next_p <= min(pi + 3, len(pair_list) - 1):
                issue_pair(next_p)
                next_p += 1
            kv_full = pair_tiles[pi]
            st = groups[gis[0]][3]
            s = sum(groups[gi][2] for gi in gis)
            ko = st - pair_st0[pair_of_call[ci]]

            # edge features, feature-major contiguous: [64, s*128]
            ea_t = sb.tile([D, SW_MAX * SUB], dt.bfloat16, tag="ea")
            nc.sync.dma_start(
                out=ea_t[:, 0:s * SUB],
                in_=eat_d[:, st * SUB:(st + s) * SUB])
            dl_t = sb2.tile([128, SW_MAX], dt.bfloat16, tag="dl")
            nc.sync.dma_start(out=dl_t[:, 0:s], in_=dlcol_d[:, st:st + s])
            # dloc free-major, replicated to 128 partitions via DMA; inner
            # dim padded to 136 so DVE reads it through the strided path
            dlb_t = sb.tile([128, SW_MAX, SUB + 8], dt.bfloat16, tag="dlb")
            nc.scalar.dma_start(
                out=dlb_t[:, 0:s, 0:SUB],
                in_=dlb_d[0:1, st * SUB:(st + s) * SUB].rearrange(
                    "p (m e) -> p m e", e=SUB).to_broadcast([128, s, SUB]))

            ef_ps = ps.tile([128, SW_MAX, D], dt.float32, tag="ef")
            for si in range(s):
                nc.tensor.matmul(
                    out=ef_ps[:, si, :],
                    lhsT=ea_t[:, si * SUB:(si + 1) * SUB],
                    rhs=we_t[:], start=True, stop=True)
            # drain Ef to SBUF on the ACT engine right away: frees the single
            # PSUM buf so the next wave's Ef matmuls don't wait on this
            # wave's vector chain
            ef_sb = sb.tile([128, SW_MAX, D], dt.bfloat16, tag="efsb")
            nc.scalar.copy(out=ef_sb[:, 0:s, :], in_=ef_ps[:, 0:s, :])

            # one-hot M [128e, s, 128n]
            m_t = sb.tile([128, SW_MAX, CHUNK], dt.bfloat16, tag="m")
            nc.vector.tensor_tensor(
                out=m_t[:, 0:s, :],
                in0=dl_t[:, 0:s].unsqueeze(2).to_broadcast([128, s, CHUNK]),
                in1=iota_t[:].unsqueeze(1).to_broadcast([128, s, CHUNK]),
                op=mybir.AluOpType.is_equal)

            # M_T [128n, s, 128e] from the partition-broadcast dloc row;
            # iotac rides the per-partition scalar operand (single-src op)
            mT_t = sb.tile([128, SW_MAX, 128], dt.bfloat16, tag="mT")
            nc.vector.tensor_scalar(
                out=mT_t[:, 0:s, :],
                in0=dlb_t[:, 0:s, 0:SUB],
                scalar1=iotac_t[:], scalar2=None,
                op0=mybir.AluOpType.is_equal)

            # Qd = M_T.T @ Qchunk (per-group chunk selects the Q slice)
            qd_ps = ps.tile([128, SW_MAX, D], dt.float32, tag="qd")
            for gi in gis:
                c_g, _, s_g, st_g = groups[gi]
                for si in range(s_g):
                    j = st_g - st + si
                    nc.tensor.matmul(out=qd_ps[:, j, :], lhsT=mT_t[:, j, :],
                                     rhs=qtab[:, c_g, :],
                                     start=True, stop=True)

            # t1 = K * Ef ; s2 = t1 * Qd   (PSUM operands ride DVE's PSUM port)
            t1_t = sb.tile([128, SW_MAX, D], dt.bfloat16, tag="t1")
            nc.vector.tensor_tensor(out=t1_t[:, 0:s, :],
                                    in0=kv_full[:, ko:ko + s, 0:D],
                                    in1=ef_sb[:, 0:s, :],
                                    op=mybir.AluOpType.mult)
            s2_t = sb.tile([128, SW_MAX, D], dt.bfloat16, tag="s2")
            nc.vector.tensor_tensor(out=s2_t[:, 0:s, :],
                                    in0=t1_t[:, 0:s, :],
                                    in1=qd_ps[:, 0:s, :],
                                    op=mybir.AluOpType.mult)

            # score
            sc_t = sb.tile([128, SW_MAX, H], dt.float32, tag="sc")
            nc.vector.tensor_reduce(
                out=sc_t[:, 0:s, :],
                in_=s2_t[:, 0:s, :].rearrange("p m (h d) -> p m h d", d=DH),
                axis=mybir.AxisListType.X, op=mybir.AluOpType.add)
            scc_t = sb.tile([128, SW_MAX, H], dt.float32, tag="scc")
            nc.vector.tensor_scalar(
                out=scc_t[:, 0:s, :], in0=sc_t[:, 0:s, :], scalar1=EXP_CLIP,
                scalar2=-EXP_CLIP, op0=mybir.AluOpType.min,
                op1=mybir.AluOpType.max)
            se_t = sb.tile([128, SW_MAX, H], dt.bfloat16, tag="se")
            nc.scalar.activation(out=se_t[:, 0:s, :], in_=scc_t[:, 0:s, :],
                                 func=mybir.ActivationFunctionType.Exp)

            # payload [128e, s, 72]
            pl_t = sb.tile([128, SW_MAX, 2 * D + H], dt.bfloat16, tag="pl")
            nc.vector.tensor_tensor(
                out=pl_t[:, 0:s, 0:D].rearrange("p m (h d) -> p m h d", d=DH),
                in0=kv_full[:, ko:ko + s, D:2 * D].rearrange(
                    "p m (h d) -> p m h d", d=DH),
                in1=se_t[:, 0:s, :].unsqueeze(3).to_broadcast(
                    [128, s, H, DH]),
                op=mybir.AluOpType.mult)
            nc.scalar.copy(out=pl_t[:, 0:s, D:D + H],
                           in_=se_t[:, 0:s, :])

            # scatter into per-chunk accumulators (two alive per pair)
            for gi in gis:
                c_g, _, s_g, st_g = groups[gi]
                if gi == first_grp[c_g]:
                    acc_tiles[c_g] = ps_acc.tile(
                        [D + H, CHUNK], dt.float32, name="chunk_acc",
                        tag=f"acc{c_g % 2}")
                accT = acc_tiles[c_g]
                for si in range(s_g):
                    j = st_g - st + si
                    nc.tensor.matmul(
                        out=accT[:],
                        lhsT=pl_t[:, j, 0:D + H],
                        rhs=m_t[:, j, :],
                        start=(gi == first_grp[c_g] and si == 0),
                        stop=(gi == last_grp[c_g] and si == s_g - 1))

                if gi == last_grp[c_g]:
                    acc_tiles.pop(c_g)
                    cp_sb = sb.tile([D + H, CHUNK], dt.float32, tag="cp")
                    nc.scalar.copy(out=cp_sb[:], in_=accT[:])
                    ot_ps = ps1.tile([CHUNK, D + H], dt.float32, tag="dlrow")
                    nc.tensor.transpose(out=ot_ps[:], in_=cp_sb[:],
                                        identity=ident[0:D + H, 0:D + H])
                    ze_t = sb.tile([CHUNK, H], dt.float32, tag="ze")
                    nc.vector.tensor_scalar_add(
                        out=ze_t[:], in0=ot_ps[:, D:D + H], scalar1=1e-6)
                    rz_t = sb.tile([CHUNK, H], dt.float32, tag="rz")
                    nc.vector.reciprocal(out=rz_t[:], in_=ze_t[:])
                    on_t = sb.tile([CHUNK, D], dt.float32, tag="on")
                    nc.vector.tensor_tensor(
                        out=on_t[:].rearrange("p (h d) -> p h d", d=DH),
                        in0=ot_ps[:, 0:D].rearrange("p (h d) -> p h d", d=DH),
                        in1=rz_t[:].unsqueeze(2).to_broadcast(
                            [CHUNK, H, DH]),
                        op=mybir.AluOpType.mult)
                    nc.sync.dma_start(
                        out=out_d[c_g * CHUNK:(c_g + 1) * CHUNK, :],
                        in_=on_t[:])
    nc.compile()
    return nc


_PROGRAM_CACHE = {}
TRACE = False
LAST_RESULTS = None
LAST_GEOM = None


def kernel(**inputs):
    x = np.asarray(inputs["x"], dtype=np.float32)
    edge_attr = np.asarray(inputs["edge_attr"], dtype=np.float32)
    WQ = np.asarray(inputs["WQ"], dtype=np.float32)
    WK = np.asarray(inputs["WK"], dtype=np.float32)
    WV = np.asarray(inputs["WV"], dtype=np.float32)
    WE = np.asarray(inputs["WE"], dtype=np.float32)
    edge_index = np.asarray(inputs["edge_index"])

    per_core, shared, geom = _preprocess(
        x, edge_attr, WQ, WK, WV, WE, edge_index)
    global LAST_GEOM
    LAST_GEOM = (per_core, shared, geom)

    key = (geom["ts"], tuple(tuple(g) for g in geom["groups"]),
           tuple(geom["calls"]))
    if key not in _PROGRAM_CACHE:
        _PROGRAM_CACHE[key] = _build_program(geom)
    nc = _PROGRAM_CACHE[key]

    in_maps = []
    for m in range(NCORES):
        im = dict(shared)
        im.update(per_core[m])
        in_maps.append({k: np.asarray(v) for k, v in im.items()})

    from concourse.bass_utils import run_bass_kernel_spmd

    res = run_bass_kernel_spmd(nc, in_maps, list(range(NCORES)), trace=TRACE)
    global LAST_RESULTS
    LAST_RESULTS = res
    out = np.empty((N, D), dtype=np.float32)
    for m in range(NCORES):
        out[m * NPC:(m + 1) * NPC] = res.results[m]["out"][:NPC]
    return out


# revision 70
# speedup vs baseline: 1.1038x; 1.1038x over previous
"""Exphormer attention (GNN message passing) Trainium2 Bass kernel.

Strategy (dst-sharded, zero collectives):
  - Core m owns nodes [m*12500, (m+1)*12500) and all edges pointing into
    them; each core computes its output slice, no collectives needed.
  - Edges are grouped by (dst-chunk of 128 nodes, src-quarter of the KV
    table), padded to 128-edge subtiles, and ordered (chunk-pair, quarter,
    chunk).  One `dma_gather` call per (chunk-pair, quarter) fetches K|V
    rows for two adjacent groups into one shared kv tile (int16 indices
    local to a 25024-row table quarter, wrapped in 16 partitions; the
    call's tail is trimmed to the worst core's valid count rounded to 16,
    skipping most pad descriptors; output is edge-major: one gathered row
    per partition, one column per 128-index block).
  - One compute wave per call: Ef = eaT @ WE on PE; one-hot
    M[e,n] = (dloc[e]==iota) on DVE; M_T via a partition-broadcast DMA of
    dloc (free-major) + a one-src tensor_scalar eq on DVE; Qd = M_T.T @
    Qchunk on PE per subtile; score = exp(clip(sum_dh K*Ef*Qd)); payload
    [e,72] = [V*score | score]; scatter = payload.T @ M accumulated in
    per-chunk PSUM tiles (two alive per pair).  PSUM operands feed DVE
    directly (separate port from the gather-contended SBUF one) and
    PSUM->SBUF casts ride the scalar (ACT) engine.
  - Chunk epilogue: copy, transpose, out = wV * recip(Z+eps), DMA
    node-major.
"""

import sys

import numpy as np

sys.path.insert(0, "/opt/trn_rl_repo")

import ml_dtypes  # noqa: E402

BF16 = ml_dtypes.bfloat16

# ---------------- problem geometry (hardcoded per contract) ----------------
N = 100000
NE = 1250000
D = 64
H = 8
DH = 8
NCORES = 8
NPC = N // NCORES          # 12500 nodes per core
CHUNK = 128                # nodes per dst-chunk
NCHUNK = (NPC + CHUNK - 1) // CHUNK   # 98
NPAD = NCHUNK * CHUNK      # 12544
NTPAD = 100096             # table rows (4 * QSIZE)
NQ = 4                     # table quarters (int16 gather index range)
QSIZE = NTPAD // NQ        # 25024 rows per quarter (< 32768)
SUB = 128                  # edges per subtile
GCALL_SUB = 8              # max subtiles per call (fw ring: 64 desc/engine)
NO_GATHER = False          # debug: replace gather with memset
EXP_CLIP = 5.0


# ---------------- host-side preprocessing ----------------
def _preprocess(x, edge_attr, WQ, WK, WV, WE, edge_index):
    src = np.ascontiguousarray(edge_index[0]).astype(np.int64)
    dst = np.ascontiguousarray(edge_index[1]).astype(np.int64)
    core_of = dst // NPC
    dloc_all = dst - core_of * NPC
    chunk_all = dloc_all // CHUNK
    quarter_all = src // QSIZE

    # order edges by (core, chunk, quarter)
    order = np.lexsort((quarter_all, chunk_all, core_of))
    src_s = src[order]
    dloc_s = (dloc_all - chunk_all * CHUNK)[order]
    key_s = (core_of * (NCHUNK * NQ) + chunk_all * NQ + quarter_all)[order]

    # counts per (core, chunk, quarter)
    cnt = np.bincount(key_s, minlength=NCORES * NCHUNK * NQ).reshape(
        NCORES, NCHUNK, NQ)
    # uniform subtile counts per (chunk, quarter): max over cores
    S = np.ceil(cnt.max(axis=0) / SUB).astype(np.int64)     # [NCHUNK, NQ]
    ts = int(S.sum())

    # group table (static program structure): one entry per (c, q) with
    # S>0, ordered (chunk-pair, quarter, chunk) so a (pair, quarter) call
    # covers two adjacent groups
    groups = []           # (c, q, s_count, sub_start)
    sub_start = 0
    for p in range(0, NCHUNK, 2):
        for q in range(NQ):
            for c in (p, p + 1):
                if c >= NCHUNK:
                    continue
                s = int(S[c, q])
                if s == 0:
                    continue
                groups.append((c, q, s, sub_start))
                sub_start += s
    assert sub_start == ts

    # one gather call per group, statically sized to the worst core's
    # valid count rounded up to 16 (idx wrap granularity) — skips most
    # of the pad-to-128 descriptors without any register plumbing
    maxcnt = cnt.max(axis=0)                                 # [NCHUNK, NQ]
    nig = []                                                 # per group
    for (c, q, s, st) in groups:
        nig.append(int(min(s * SUB,
                           (max(int(maxcnt[c, q]), 1) + 15) // 16 * 16)))

    # calls: one per (chunk-pair, quarter).  Interior groups stay padded
    # to s*128 (slot alignment); the call's last group trims to its nig.
    calls = []            # (q, [gi...], n_idx)
    i = 0
    while i < len(groups):
        c, q, s, st = groups[i]
        gl = [i]
        if i + 1 < len(groups):
            c2, q2 = groups[i + 1][0], groups[i + 1][1]
            if q2 == q and c2 // 2 == c // 2:
                gl.append(i + 1)
        n_idx = sum(groups[g][2] * SUB for g in gl[:-1]) + nig[gl[-1]]
        calls.append((q, gl, n_idx))
        i = gl[-1] + 1
    idx_cols = [n // 16 for (q, gl, n) in calls]
    idx_col_start = np.concatenate([[0], np.cumsum(idx_cols)]).astype(int)
    total_idx_cols = int(idx_col_start[-1])

    geom = dict(ts=ts, groups=groups, nig=nig,
                calls=[(q, tuple(gl), n) for (q, gl, n) in calls],
                idx_col_start=idx_col_start,
                total_idx_cols=total_idx_cols, S=S)

    # ---- per-core data staging ----
    core_starts = np.searchsorted(key_s // (NCHUNK * NQ), np.arange(NCORES + 1))
    per_core = []
    for m in range(NCORES):
        lo, hi = core_starts[m], core_starts[m + 1]
        k_loc = key_s[lo:hi] - m * (NCHUNK * NQ)     # chunk*NQ + quarter
        c_src = src_s[lo:hi]
        c_dloc = dloc_s[lo:hi]
        c_eid = order[lo:hi]
        grp_starts = np.searchsorted(k_loc, np.arange(NCHUNK * NQ + 1))

        E_pad = ts * SUB
        srcq_pad = np.zeros(E_pad, dtype=np.int16)   # quarter-local idx
        dloc_pad = np.full(E_pad, -1.0, dtype=np.float32)
        eid_pad = np.full(E_pad, -1, dtype=np.int64)
        for gi, (c, q, s, st) in enumerate(groups):
            a, b = grp_starts[c * NQ + q], grp_starts[c * NQ + q + 1]
            n = b - a
            pos = st * SUB
            srcq_pad[pos:pos + n] = (c_src[a:b] - q * QSIZE).astype(np.int16)
            dloc_pad[pos:pos + n] = c_dloc[a:b]
            eid_pad[pos:pos + n] = c_eid[a:b]

        valid = eid_pad >= 0
        ea_pad = np.zeros((E_pad, D), dtype=BF16)
        ea_pad[valid] = edge_attr[eid_pad[valid]].astype(BF16)
        # [64, ts*128]: per-subtile feature-major, contiguous per group
        eat = np.ascontiguousarray(
            ea_pad.reshape(ts, SUB, D).transpose(2, 0, 1).reshape(
                D, ts * SUB))
        dlcol = np.ascontiguousarray(
            dloc_pad.reshape(ts, SUB).T).astype(BF16)            # [128,ts]
        dlb = dloc_pad.reshape(1, ts * SUB).astype(BF16)         # [1,ts*128]

        # gather idx stream per call: wrapped in 16 partitions,
        # replicated to all 128: idxarr[16g + j, col] = stream pos col*16+j.
        idxarr = np.zeros((128, total_idx_cols), dtype=np.int16)
        for ci, (q, gl, n_idx) in enumerate(calls):
            col0 = int(idx_col_start[ci])
            parts = []
            for k, gi in enumerate(gl):
                st_g, s_g = groups[gi][3], groups[gi][2]
                take = nig[gi] if k == len(gl) - 1 else s_g * SUB
                parts.append(srcq_pad[st_g * SUB:st_g * SUB + take])
            stream = np.concatenate(parts)
            assert stream.size == n_idx
            wrapped = stream.reshape(-1, 16).T                   # [16, cols]
            idxarr[:, col0:col0 + wrapped.shape[1]] = np.tile(wrapped, (8, 1))

        n0 = m * NPC
        xq = np.zeros((NPAD, D), dtype=np.float32)
        xq[:NPC] = x[n0:n0 + NPC]
        xtq = np.ascontiguousarray(xq.T).astype(BF16)

        per_core.append(dict(eat=eat, dlcol=dlcol, dlb=dlb, idxarr=idxarr,
                             xtq=xtq))

    xt_full = np.zeros((D, NTPAD), dtype=BF16)
    xt_full[:, :N] = x.T.astype(BF16)
    wkv = np.concatenate([WK, WV], axis=1).astype(BF16)
    wq = (WQ / np.sqrt(DH)).astype(BF16)
    we = WE.astype(BF16)
    iota_row = np.ascontiguousarray(np.broadcast_to(
        np.arange(CHUNK, dtype=np.float32), (128, CHUNK)).astype(BF16))
    iota_col = np.arange(128, dtype=np.float32).reshape(128, 1)

    shared = dict(xt=xt_full, wkv=wkv, wq=wq, we=we, iota_row=iota_row,
                  iota_col=iota_col)
    return per_core, shared, geom


# ---------------- device program ----------------
def _build_program(geom):
    from contextlib import ExitStack

    from concourse import bacc, mybir
    import concourse.tile as tile
    from concourse.masks import make_identity
    from concourse.tile_rust import add_dep_helper

    n_table_rows = NTPAD
    nchunk = NCHUNK
    ts = geom["ts"]
    groups = geom["groups"]
    calls = geom["calls"]
    idx_col_start = geom["idx_col_start"]
    total_idx_cols = geom["total_idx_cols"]
    S_MAX = max(g[2] for g in groups)

    dt = mybir.dt
    nc = bacc.Bacc("TRN2", target_bir_lowering=False, debug=False,
                   num_devices=NCORES, dynamic_dma_scratch_size=32768)

    xt = nc.dram_tensor("xt", [D, n_table_rows], dt.bfloat16,
                        kind="ExternalInput").ap()
    xtq = nc.dram_tensor("xtq", [D, nchunk * CHUNK], dt.bfloat16,
                         kind="ExternalInput").ap()
    wkv_d = nc.dram_tensor("wkv", [D, 2 * D], dt.bfloat16,
                           kind="ExternalInput").ap()
    wq_d = nc.dram_tensor("wq", [D, D], dt.bfloat16, kind="ExternalInput").ap()
    we_d = nc.dram_tensor("we", [D, D], dt.bfloat16, kind="ExternalInput").ap()
    iota_d = nc.dram_tensor("iota_row", [128, CHUNK], dt.bfloat16,
                            kind="ExternalInput").ap()
    iotac_d = nc.dram_tensor("iota_col", [128, 1], dt.float32,
                             kind="ExternalInput").ap()
    eat_d = nc.dram_tensor("eat", [D, ts * SUB], dt.bfloat16,
                           kind="ExternalInput").ap()
    idx_d = nc.dram_tensor("idxarr", [128, total_idx_cols], dt.int16,
                           kind="ExternalInput").ap()
    dlcol_d = nc.dram_tensor("dlcol", [128, ts], dt.bfloat16,
                             kind="ExternalInput").ap()
    dlb_d = nc.dram_tensor("dlb", [1, ts * SUB], dt.bfloat16,
                           kind="ExternalInput").ap()
    out_d = nc.dram_tensor("out", [nchunk * CHUNK, D], dt.float32,
                           kind="ExternalOutput").ap()
    kvtab = nc.dram_tensor("kvtab", [n_table_rows, 2 * D], dt.bfloat16).ap()

    with tile.TileContext(nc) as tc, ExitStack() as ctx:
        const_p = ctx.enter_context(tc.tile_pool(name="const", bufs=1))
        sb = ctx.enter_context(tc.tile_pool(name="sb", bufs=4))
        sb2 = ctx.enter_context(tc.tile_pool(name="sb2", bufs=2))
        gat = ctx.enter_context(tc.tile_pool(name="gat", bufs=3))
        ps = ctx.enter_context(tc.tile_pool(name="ps", bufs=1, space="PSUM"))
        ps1 = ctx.enter_context(tc.tile_pool(name="ps1", bufs=1, space="PSUM"))
        ps_acc = ctx.enter_context(
            tc.tile_pool(name="ps_acc", bufs=1, space="PSUM"))

        ident = const_p.tile([128, 128], dt.float32)
        make_identity(nc, ident[:])
        wkv_t = const_p.tile([D, 2 * D], dt.bfloat16)
        nc.sync.dma_start(out=wkv_t[:], in_=wkv_d)
        wq_t = const_p.tile([D, D], dt.bfloat16)
        nc.sync.dma_start(out=wq_t[:], in_=wq_d)
        we_t = const_p.tile([D, D], dt.bfloat16)
        nc.sync.dma_start(out=we_t[:], in_=we_d)
        iota_t = const_p.tile([128, CHUNK], dt.bfloat16)
        nc.sync.dma_start(out=iota_t[:], in_=iota_d)
        iotac_t = const_p.tile([128, 1], dt.float32)
        nc.sync.dma_start(out=iotac_t[:], in_=iotac_d)

        # ---- pre-pass 1: Q table resident in SBUF (first, so the main
        # waves aren't gated on the KV-table build) ----
        qtab = const_p.tile([128, nchunk, D], dt.bfloat16)
        for c0 in range(0, nchunk, 4):
            nq_blk = min(4, nchunk - c0)
            xq_t = sb.tile([D, 4 * 128], dt.bfloat16, tag="xq_t")
            nc.sync.dma_start(out=xq_t[:, 0:nq_blk * 128],
                              in_=xtq[:, c0 * 128:(c0 + nq_blk) * 128])
            for bi in range(nq_blk):
                c = c0 + bi
                q_ps = ps.tile([128, D], dt.float32, name="q_ps",
                               tag="qd" if c % 2 == 0 else "ef")
                nc.tensor.matmul(out=q_ps[:],
                                 lhsT=xq_t[:, bi * 128:(bi + 1) * 128],
                                 rhs=wq_t[:], start=True, stop=True)
                nc.scalar.copy(out=qtab[:, c, :], in_=q_ps[:])

        # ---- pre-pass 2: KV table -> DRAM (batched loads + stores of 4
        # blocks; gathers gate on the store covering their quarter) ----
        n_tb = n_table_rows // 128
        stores = []
        for b0 in range(0, n_tb, 4):
            nblk = min(4, n_tb - b0)
            xt_t = sb.tile([D, 4 * 128], dt.bfloat16, tag="xt_t")
            nc.sync.dma_start(out=xt_t[:, 0:nblk * 128],
                              in_=xt[:, b0 * 128:(b0 + nblk) * 128])
            kv_sb = sb.tile([128, 4, 2 * D], dt.bfloat16, tag="kv_sb")
            for bi in range(nblk):
                # alternate psum tags: double-buffers the prepass matmuls
                kv_ps = ps.tile([128, 2 * D], dt.float32, name="kv_ps",
                                tag="ef" if bi % 2 == 0 else "qd")
                nc.tensor.matmul(out=kv_ps[:],
                                 lhsT=xt_t[:, bi * 128:(bi + 1) * 128],
                                 rhs=wkv_t[:], start=True, stop=True)
                nc.scalar.copy(out=kv_sb[:, bi, :], in_=kv_ps[:])
            stores.append(nc.sync.dma_start(
                out=kvtab[b0 * 128:(b0 + nblk) * 128, :].rearrange(
                    "(blk p) d -> p blk d", p=128),
                in_=kv_sb[:, 0:nblk, :]))
        # store index whose coverage reaches each quarter's end
        qgate = [min((QSIZE * (qq + 1) + 511) // 512 - 1, len(stores) - 1)
                 for qq in range(NQ)]

        # ---- main loop: one kv tile per chunk-pair, one gather call and
        # one compute wave per (pair, quarter) ----
        pair_of_call = [groups[gl[0]][0] // 2 for (q, gl, n) in calls]
        pair_calls = {}
        for ci, p in enumerate(pair_of_call):
            pair_calls.setdefault(p, []).append(ci)
        pair_list = sorted(pair_calls)
        pair_idx = {p: i for i, p in enumerate(pair_list)}
        pair_st0 = {p: groups[calls[pair_calls[p][0]][1][0]][3]
                    for p in pair_list}
        SP_MAX = max(sum(groups[gi][2] for ci in pair_calls[p]
                         for gi in calls[ci][1]) for p in pair_list)
        SW_MAX = max(sum(groups[gi][2] for gi in gl) for (q, gl, n) in calls)
        S_CAP = SW_MAX

        first_grp = {}
        last_grp = {}
        for gi, (c, q, s, st) in enumerate(groups):
            if c not in first_grp:
                first_grp[c] = gi
            last_grp[c] = gi

        pair_tiles = {}

        def issue_pair(pi):
            p = pair_list[pi]
            st0 = pair_st0[p]
            kv_t = gat.tile([128, SP_MAX, 2 * D], dt.bfloat16, tag="kvq")
            if pi < 3:
                # first pool rotations: clear garbage (NaN-safety); later
                # rotations inherit finite stale values.  ACT engine: idle
                # during startup, keeps DVE off the critical path
                nc.scalar.memzero(kv_t[:])
            for ci in pair_calls[p]:
                q, gl, n_idx = calls[ci]
                off = groups[gl[0]][3] - st0
                s_call = sum(groups[gi][2] for gi in gl)
                col0 = int(idx_col_start[ci])
                col1 = int(idx_col_start[ci + 1])
                idx_t = sb2.tile([128, S_CAP * 8], dt.int16, tag="idx",
                                 bufs=12)
                nc.scalar.dma_start(out=idx_t[:, 0:col1 - col0],
                                    in_=idx_d[:, col0:col1])
                g = nc.gpsimd.dma_gather(
                    out_ap=kv_t[:, off:off + s_call, :],
                    in_ap=kvtab[q * QSIZE:(q + 1) * QSIZE, :],
                    idxs_ap=idx_t[:, 0:col1 - col0],
                    num_idxs=n_idx,
                    num_idxs_reg=n_idx,
                    elem_size=2 * D,
                )
                add_dep_helper(g.ins, stores[qgate[q]].ins, True,
                               "gather after its kv quarter is built")
                if qgate[q] > 0:
                    # also gate on the preceding store: covers completion
                    # skew between in-flight store transfers
                    add_dep_helper(g.ins, stores[qgate[q] - 1].ins, True,
                                   "gather after prior kv store")
            pair_tiles[pi] = kv_t

        issue_pair(0)
        next_p = 1
        acc_tiles = {}
        for ci, (wq, gis, n_idx) in enumerate(calls):
            pi = pair_idx[pair_of_call[ci]]
            while next_p <= min(pi + 2, len(pair_list) - 1):
                issue_pair(next_p)
                next_p += 1
            kv_full = pair_tiles[pi]
            st = groups[gis[0]][3]
            s = sum(groups[gi][2] for gi in gis)
            ko = st - pair_st0[pair_of_call[ci]]

            # edge features, feature-major contiguous: [64, s*128]
            ea_t = sb.tile([D, SW_MAX * SUB], dt.bfloat16, tag="ea")
            nc.sync.dma_start(
                out=ea_t[:, 0:s * SUB],
                in_=eat_d[:, st * SUB:(st + s) * SUB])
            dl_t = sb2.tile([128, SW_MAX], dt.bfloat16, tag="dl")
            nc.sync.dma_start(out=dl_t[:, 0:s], in_=dlcol_d[:, st:st + s])
            # dloc free-major, replicated to 128 partitions via DMA; inner
            # dim padded to 136 so DVE reads it through the strided path
            dlb_t = sb.tile([128, SW_MAX, SUB + 8], dt.bfloat16, tag="dlb")
            nc.scalar.dma_start(
                out=dlb_t[:, 0:s, 0:SUB],
                in_=dlb_d[0:1, st * SUB:(st + s) * SUB].rearrange(
                    "p (m e) -> p m e", e=SUB).to_broadcast([128, s, SUB]))

            ef_ps = ps.tile([128, SW_MAX, D], dt.float32, tag="ef")
            for si in range(s):
                nc.tensor.matmul(
                    out=ef_ps[:, si, :],
                    lhsT=ea_t[:, si * SUB:(si + 1) * SUB],
                    rhs=we_t[:], start=True, stop=True)
            # drain Ef to SBUF on the ACT engine right away: frees the single
            # PSUM buf so the next wave's Ef matmuls don't wait on this
            # wave's vector chain
            ef_sb = sb.tile([128, SW_MAX, D], dt.bfloat16, tag="efsb")
            nc.scalar.copy(out=ef_sb[:, 0:s, :], in_=ef_ps[:, 0:s, :])

            # one-hot M [128e, s, 128n]
            m_t = sb.tile([128, SW_MAX, CHUNK], dt.bfloat16, tag="m")
            nc.vector.tensor_tensor(
                out=m_t[:, 0:s, :],
                in0=dl_t[:, 0:s].unsqueeze(2).to_broadcast([128, s, CHUNK]),
                in1=iota_t[:].unsqueeze(1).to_broadcast([128, s, CHUNK]),
                op=mybir.AluOpType.is_equal)

            # M_T [128n, s, 128e] from the partition-broadcast dloc row;
            # iotac rides the per-partition scalar operand (single-src op)
            mT_t = sb.tile([128, SW_MAX, 128], dt.bfloat16, tag="mT")
            nc.vector.tensor_scalar(
                out=mT_t[:, 0:s, :],
                in0=dlb_t[:, 0:s, 0:SUB],
                scalar1=iotac_t[:], scalar2=None,
                op0=mybir.AluOpType.is_equal)

            # Qd = M_T.T @ Qchunk (per-group chunk selects the Q slice)
            qd_ps = ps.tile([128, SW_MAX, D], dt.float32, tag="qd")
            for gi in gis:
                c_g, _, s_g, st_g = groups[gi]
                for si in range(s_g):
                    j = st_g - st + si
                    nc.tensor.matmul(out=qd_ps[:, j, :], lhsT=mT_t[:, j, :],
                                     rhs=qtab[:, c_g, :],
                                     start=True, stop=True)

            # t1 = K * Ef ; s2 = t1 * Qd   (PSUM operands ride DVE's PSUM port)
            t1_t = sb.tile([128, SW_MAX, D], dt.bfloat16, tag="t1")
            nc.vector.tensor_tensor(out=t1_t[:, 0:s, :],
                                    in0=kv_full[:, ko:ko + s, 0:D],
                                    in1=ef_sb[:, 0:s, :],
                                    op=mybir.AluOpType.mult)
            s2_t = sb.tile([128, SW_MAX, D], dt.bfloat16, tag="s2")
            nc.vector.tensor_tensor(out=s2_t[:, 0:s, :],
                                    in0=t1_t[:, 0:s, :],
                                    in1=qd_ps[:, 0:s, :],
                                    op=mybir.AluOpType.mult)

            # score
            sc_t = sb.tile([128, SW_MAX, H], dt.float32, tag="sc")
            nc.vector.tensor_reduce(
                out=sc_t[:, 0:s, :],
                in_=s2_t[:, 0:s, :].rearrange("p m (h d) -> p m h d", d=DH),
                axis=mybir.AxisListType.X, op=mybir.AluOpType.add)
            scc_t = sb.tile([128, SW_MAX, H], dt.float32, tag="scc")
            nc.vector.tensor_scalar(
                out=scc_t[:, 0:s, :], in0=sc_t[:, 0:s, :], scalar1=EXP_CLIP,
                scalar2=-EXP_CLIP, op0=mybir.AluOpType.min,
                op1=mybir.AluOpType.max)
            se_t = sb.tile([128, SW_MAX, H], dt.bfloat16, tag="se")
            nc.scalar.activation(out=se_t[:, 0:s, :], in_=scc_t[:, 0:s, :],
                                 func=mybir.ActivationFunctionType.Exp)

            # payload [128e, s, 72]
            pl_t = sb.tile([128, SW_MAX, 2 * D + H], dt.bfloat16, tag="pl")
            nc.vector.tensor_tensor(
                out=pl_t[:, 0:s, 0:D].rearrange("p m (h d) -> p m h d", d=DH),
                in0=kv_full[:, ko:ko + s, D:2 * D].rearrange(
                    "p m (h d) -> p m h d", d=DH),
                in1=se_t[:, 0:s, :].unsqueeze(3).to_broadcast(
                    [128, s, H, DH]),
                op=mybir.AluOpType.mult)
            nc.scalar.copy(out=pl_t[:, 0:s, D:D + H],
                           in_=se_t[:, 0:s, :])

            # scatter into per-chunk accumulators (two alive per pair)
            for gi in gis:
                c_g, _, s_g, st_g = groups[gi]
                if gi == first_grp[c_g]:
                    acc_tiles[c_g] = ps_acc.tile(
                        [D + H, CHUNK], dt.float32, name="chunk_acc",
                        tag=f"acc{c_g % 2}")
                accT = acc_tiles[c_g]
                for si in range(s_g):
                    j = st_g - st + si
                    nc.tensor.matmul(
                        out=accT[:],
                        lhsT=pl_t[:, j, 0:D + H],
                        rhs=m_t[:, j, :],
                        start=(gi == first_grp[c_g] and si == 0),
                        stop=(gi == last_grp[c_g] and si == s_g - 1))

                if gi == last_grp[c_g]:
                    acc_tiles.pop(c_g)
                    cp_sb = sb.tile([D + H, CHUNK], dt.float32, tag="cp")
                    nc.scalar.copy(out=cp_sb[:], in_=accT[:])
                    ot_ps = ps1.tile([CHUNK, D + H], dt.float32, tag="dlrow")
                    nc.tensor.transpose(out=ot_ps[:], in_=cp_sb[:],
                                        identity=ident[0:D + H, 0:D + H])
                    ze_t = sb.tile([CHUNK, H], dt.float32, tag="ze")
                    nc.vector.tensor_scalar_add(
                        out=ze_t[:], in0=ot_ps[:, D:D + H], scalar1=1e-6)
                    rz_t = sb.tile([CHUNK, H], dt.float32, tag="rz")
                    nc.vector.reciprocal(out=rz_t[:], in_=ze_t[:])
                    on_t = sb.tile([CHUNK, D], dt.float32, tag="on")
                    nc.vector.tensor_tensor(
                        out=on_t[:].rearrange("p (h d) -> p h d", d=DH),
                        in0=ot_ps[:, 0:D].rearrange("p (h d) -> p h d", d=DH),
                        in1=rz_t[:].unsqueeze(2).to_broadcast(
                            [CHUNK, H, DH]),
                        op=mybir.AluOpType.mult)
                    nc.sync.dma_start(
                        out=out_d[c_g * CHUNK:(c_g + 1) * CHUNK, :],
                        in_=on_t[:])
    nc.compile()
    return nc


_PROGRAM_CACHE = {}
TRACE = False
LAST_RESULTS = None
LAST_GEOM = None


def kernel(**inputs):
    x = np.asarray(inputs["x"], dtype=np.float32)
    edge_attr = np.asarray(inputs["edge_attr"], dtype=np.float32)
    WQ = np.asarray(inputs["WQ"], dtype=np.float32)
    WK = np.asarray(inputs["WK"], dtype=np.float32)
    WV = np.asarray(inputs["WV"], dtype=np.float32)
    WE = np.asarray(inputs["WE"], dtype=np.float32)
    edge_index = np.asarray(inputs["edge_index"])

    per_core, shared, geom = _preprocess(
        x, edge_attr, WQ, WK, WV, WE, edge_index)
    global LAST_GEOM
    LAST_GEOM = (per_core, shared, geom)

    key = (geom["ts"], tuple(tuple(g) for g in geom["groups"]),
           tuple(geom["calls"]))
    if key not in _PROGRAM_CACHE:
        _PROGRAM_CACHE[key] = _build_program(geom)
    nc = _PROGRAM_CACHE[key]

    in_maps = []
    for m in range(NCORES):
        im = dict(shared)
        im.update(per_core[m])
        in_maps.append({k: np.asarray(v) for k, v in im.items()})

    from concourse.bass_utils import run_bass_kernel_spmd

    res = run_bass_kernel_spmd(nc, in_maps, list(range(NCORES)), trace=TRACE)
    global LAST_RESULTS
    LAST_RESULTS = res
    out = np.empty((N, D), dtype=np.float32)
    for m in range(NCORES):
        out[m * NPC:(m + 1) * NPC] = res.results[m]["out"][:NPC]
    return out
